# revision 24
# baseline (speedup 1.0000x reference)
"""SIR ODE batch integrator on 8 Trainium2 NeuronCores (Bass/Tile).

Problem: for each of B=65536 samples with params (beta, gamma, S0, I0),
integrate dS=-bSI, dI=bSI-gI, dR=gI over 199 fixed intervals
(t = linspace(0,100,200), fp32) and return the trajectory [B, 200, 3].

Strategy:
  - Pure data parallel: 8192 samples per core as [128 part, 64 free].
  - Scaled 2-state formulation: w = beta*S, ct = beta*C (C = S+I).
      dw/dt = -w*v,  dct/dt = -gamma*v,   v = ct - w  (= beta*I)
    so the derivative X = [gamma*v | w*v] is TWO plain elementwise ops
    (v = ct - w; X = (vv*a) * [gamma|w]) - no custom DVE op - and the
    state update is DIAGONAL (plain subtract).  Host recovers
    S = w/beta, I = v/beta, R = 1 - ct/beta.
  - Schedule (fp32-validated vs the reference: rel fro-norm 1.36e-3,
    gate 2e-2):
      k 0..3    RK4, two half-width sample groups interleaved so each
                group's op latency hides under the other group's ops
      k 4..7    midpoint RK2, same interleave
      k 8       midpoint step that seeds the AB2 history + Z init
      k 9..29   Z-form AB2, single full-width chain: with
                Xs_n = (3/2)dt_n*X_n and Z_n = Y_n + (1/3)Xs_{n-1},
                  Y_{n+1} = Z_n - Xs_n         (cycle: v -> Xs -> Y)
                  Z_{n+1} = Z_n - (2/3)Xs_n    (fills the Y->v gap)
      k 30..198 multiplicative forward Euler on state [w | m], m = -v:
                  qr = state*dtbar + [G|1],  state' = state*swap(qr)
                (G = 1 - dtbar*gamma).  TWO DVE ops per interval; two
                half-width groups interleaved -> busy-bound
                ~510ns/interval with no exposed latency hops.
  - gamma rides at column 0 of each staging slab so the X op can read
    [gamma | w] as a single two-block access pattern of one tensor.
  - Output: states are written in-place into [128, 64+16*128] staging
    slabs (2, ping-pong); one 1MB DMA per 16 intervals (13 DMAs total).
"""

import numpy as np

try:
    import concourse.bass as bass
except ImportError:  # pragma: no cover - container default location
    import sys

    sys.path.insert(0, "/opt/trn_rl_repo")
    import concourse.bass as bass

import concourse.bacc as bacc
import concourse.mybir as mybir
from concourse.ap import AP
from concourse.tile import TileContext
from concourse.bass_utils import run_bass_kernel_spmd

F32 = mybir.dt.float32
AL = mybir.AluOpType

N_CORES = 8
B = 65536
PER = B // N_CORES  # 8192 samples per core
P = 128
F = PER // P  # 64
NUM_T = 200
NI = NUM_T - 1  # 199 intervals
CH = 16  # intervals per output chunk (one DMA each)
NCHUNK = (NI + CH - 1) // CH  # 13 (last chunk has 7)
N_RK4 = 4  # RK4 head intervals
K_EULER = 20  # forward-Euler tail from this interval (validated: rel 1.69e-3)
N_MID = 4  # midpoint (RK2) head intervals after the RK4 block
K_SEED = N_RK4 + N_MID  # midpoint interval that seeds the AB2 history

# Bit-exact fp32 dt values of jnp.linspace(0, 100, 200, float32) diffs.
_DT_BITS = [
    0x3F00A4AA, 0x3F00A4AA, 0x3F00A4AA, 0x3F00A4AA, 0x3F00A4A8, 0x3F00A4AC, 0x3F00A4AC, 0x3F00A4A8, 0x3F00A4A8, 0x3F00A4A8,
    0x3F00A4B0, 0x3F00A4A8, 0x3F00A4A8, 0x3F00A4B0, 0x3F00A4A8, 0x3F00A4A8, 0x3F00A4B0, 0x3F00A4A0, 0x3F00A4B0, 0x3F00A4A0,
    0x3F00A4B0, 0x3F00A4B0, 0x3F00A4A0, 0x3F00A4B0, 0x3F00A4B0, 0x3F00A4A0, 0x3F00A4B0, 0x3F00A4B0, 0x3F00A4A0, 0x3F00A4B0,
    0x3F00A4A0, 0x3F00A4B0, 0x3F00A4A0, 0x3F00A4C0, 0x3F00A4A0, 0x3F00A4A0, 0x3F00A4C0, 0x3F00A4A0, 0x3F00A4A0, 0x3F00A4A0,
    0x3F00A4C0, 0x3F00A4A0, 0x3F00A4A0, 0x3F00A4C0, 0x3F00A4A0, 0x3F00A4A0, 0x3F00A4C0, 0x3F00A4A0, 0x3F00A4A0, 0x3F00A4C0,
    0x3F00A4A0, 0x3F00A4A0, 0x3F00A4C0, 0x3F00A4A0, 0x3F00A4A0, 0x3F00A4C0, 0x3F00A4A0, 0x3F00A4A0, 0x3F00A4A0, 0x3F00A4C0,
    0x3F00A4A0, 0x3F00A4A0, 0x3F00A4C0, 0x3F00A4A0, 0x3F00A4C0, 0x3F00A480, 0x3F00A4C0, 0x3F00A4C0, 0x3F00A480, 0x3F00A4C0,
    0x3F00A4C0, 0x3F00A480, 0x3F00A4C0, 0x3F00A4C0, 0x3F00A480, 0x3F00A4C0, 0x3F00A4C0, 0x3F00A480, 0x3F00A4C0, 0x3F00A480,
    0x3F00A4C0, 0x3F00A4C0, 0x3F00A480, 0x3F00A4C0, 0x3F00A4C0, 0x3F00A480, 0x3F00A4C0, 0x3F00A4C0, 0x3F00A480, 0x3F00A4C0,
    0x3F00A4C0, 0x3F00A480, 0x3F00A4C0, 0x3F00A4C0, 0x3F00A480, 0x3F00A4C0, 0x3F00A4C0, 0x3F00A480, 0x3F00A4C0, 0x3F00A4C0,
    0x3F00A480, 0x3F00A4C0, 0x3F00A4C0, 0x3F00A480, 0x3F00A4C0, 0x3F00A4C0, 0x3F00A480, 0x3F00A4C0, 0x3F00A4C0, 0x3F00A480,
    0x3F00A4C0, 0x3F00A4C0, 0x3F00A480, 0x3F00A4C0, 0x3F00A480, 0x3F00A4C0, 0x3F00A4C0, 0x3F00A480, 0x3F00A4C0, 0x3F00A4C0,
    0x3F00A480, 0x3F00A4C0, 0x3F00A4C0, 0x3F00A480, 0x3F00A4C0, 0x3F00A4C0, 0x3F00A480, 0x3F00A4C0, 0x3F00A480, 0x3F00A500,
    0x3F00A480, 0x3F00A480, 0x3F00A500, 0x3F00A480, 0x3F00A480, 0x3F00A500, 0x3F00A480, 0x3F00A480, 0x3F00A500, 0x3F00A480,
    0x3F00A480, 0x3F00A500, 0x3F00A480, 0x3F00A480, 0x3F00A500, 0x3F00A480, 0x3F00A480, 0x3F00A500, 0x3F00A480, 0x3F00A480,
    0x3F00A500, 0x3F00A480, 0x3F00A480, 0x3F00A500, 0x3F00A480, 0x3F00A480, 0x3F00A500, 0x3F00A480, 0x3F00A480, 0x3F00A480,
    0x3F00A500, 0x3F00A480, 0x3F00A480, 0x3F00A500, 0x3F00A480, 0x3F00A480, 0x3F00A500, 0x3F00A480, 0x3F00A480, 0x3F00A500,
    0x3F00A480, 0x3F00A480, 0x3F00A500, 0x3F00A480, 0x3F00A480, 0x3F00A500, 0x3F00A480, 0x3F00A480, 0x3F00A500, 0x3F00A480,
    0x3F00A480, 0x3F00A500, 0x3F00A480, 0x3F00A480, 0x3F00A500, 0x3F00A480, 0x3F00A480, 0x3F00A500, 0x3F00A480, 0x3F00A480,
    0x3F00A500, 0x3F00A480, 0x3F00A480, 0x3F00A500, 0x3F00A480, 0x3F00A480, 0x3F00A500, 0x3F00A480, 0x3F00A480,
]
DTS = np.array(_DT_BITS, dtype=np.uint32).view(np.float32)
assert DTS.shape == (NI,)

AS = [float(np.float32(1.5) * DTS[k]) for k in range(NI)]  # AB2 scale a_k
THIRD = float(np.float32(1.0 / 3.0))
TWO_THIRD = float(np.float32(2.0 / 3.0))
DTBAR = float(np.float32(np.float64(100.0) / 199.0))  # Euler-tail step

SLAB_COLS = F + CH * 2 * F  # gamma block + CH state slices


def _two_block(slab_ap, off2, sub=0, width=F):
    """AP reading [block at column sub | block at column off2+sub] of a slab
    (width columns each): free dims [[off2, 2], [1, width]]."""
    return AP(
        tensor=slab_ap.tensor,
        offset=slab_ap.offset + sub,
        ap=[list(slab_ap.ap[0]), [off2, 2], [1, width]],
    )


def _vv(v_ap):
    """[v | v] broadcast read of a [P, F] tile."""
    return v_ap.unsqueeze(1).broadcast_to([P, 2, F])


def _3d(ap2d):
    """View a [P, 2F] AP as [P, 2, F] (to match broadcast operands)."""
    return ap2d.rearrange("p (two f) -> p two f", two=2)


def build_nc(reps=1):
    # Bacc (not raw Bass): its compile() pipeline runs generate_event_semaphores,
    # which splits multi-wait sync conditions that TRN2 instructions can't carry.
    nc = bacc.Bacc(None)
    pin = nc.declare_dram_parameter("pin", [P, 3 * F], F32, isOutput=False)
    out = nc.declare_dram_parameter("out", [NCHUNK, P, CH * 2 * F], F32, isOutput=True)
    v = nc.vector

    with TileContext(nc) as tc:
        with (
            tc.tile_pool(name="const", bufs=1) as cpool,
            tc.tile_pool(name="slab", bufs=1) as spool,
            tc.tile_pool(name="work", bufs=2) as wpool,
        ):

            def body(_=None):
                pint = cpool.tile([P, 3 * F], F32, tag="pin")
                nc.sync.dma_start(out=pint[:], in_=pin[:])
                slabA = spool.tile([P, SLAB_COLS], F32, tag="slabA")
                slabB = spool.tile([P, SLAB_COLS], F32, tag="slabB")
                slabs = [slabA, slabB]
                # gamma block at column 0 of both slabs
                for s in slabs:
                    nc.sync.dma_start(out=s[:, 0:F], in_=pin[:, 0:F])

                def slice2F(k):
                    """State slice [ct | w] for interval k (2F wide)."""
                    s = slabs[(k // CH) % 2]
                    base = F + (k % CH) * 2 * F
                    return s, s[:, base : base + 2 * F], base

                def eval_X(src_slab, base, scale, xt_tag):
                    """v = ct - w; X = (vv*scale) * [gamma | w].  X layout
                    [X_ct | X_w] matching the [ct | w] state slices."""
                    vt = wpool.tile([P, F], F32, tag="v")
                    v.tensor_tensor(
                        vt[:],
                        src_slab[:, base : base + F],
                        src_slab[:, base + F : base + 2 * F],
                        AL.subtract,
                    )
                    xt = wpool.tile([P, 2 * F], F32, tag=xt_tag)
                    v.scalar_tensor_tensor(
                        _3d(xt[:]),
                        _vv(vt[:]),
                        scale,
                        _two_block(src_slab[:], base + F),
                        AL.mult,
                        AL.mult,
                    )
                    return xt

                # scratch slices in the OTHER slab (idle until chunk 1)
                def scratch(j):
                    s = slabs[1]
                    base = F + j * 2 * F
                    return s, s[:, base : base + 2 * F], base

                # --- head: RK4, intervals 0..N_RK4-1, two half-width groups
                # interleaved so dependency latency hides under the other
                # group's ops ---
                HW = F // 2  # 32 cols per group

                def g_state(slab_t, base, g):
                    """[ct_g | w_g] two-block view of a state slice."""
                    return _two_block(slab_t[:], F, sub=base + g * HW, width=HW)

                def g_gw(slab_t, base, g):
                    """[gamma_g | w_g] two-block view (gamma at slab col 0)."""
                    return _two_block(slab_t[:], base + F, sub=g * HW, width=HW)

                def head_eval_X(src_slab, base, tag):
                    return head_eval_X2(src_slab, base, 1.0, tag)

                def head_eval_X2(src_slab, base, scale, tag):
                    """Per-group derivative: returns [XA, XB] ([P,2,HW] tiles)."""
                    vts, xts = [], []
                    for g in range(2):
                        vt = wpool.tile([P, HW], F32, tag=f"v{g}")
                        v.tensor_tensor(
                            vt[:],
                            src_slab[:, base + g * HW : base + (g + 1) * HW],
                            src_slab[:, base + F + g * HW : base + F + (g + 1) * HW],
                            AL.subtract,
                        )
                        vts.append(vt)
                    for g in range(2):
                        xt = wpool.tile([P, 2, HW], F32, tag=f"{tag}{g}")
                        v.scalar_tensor_tensor(
                            xt[:],
                            vts[g][:].unsqueeze(1).broadcast_to([P, 2, HW]),
                            scale,
                            g_gw(src_slab, base, g),
                            AL.mult,
                            AL.mult,
                        )
                        xts.append(xt)
                    return xts

                def head_stt(outs, in0s, scalar, in1s):
                    for g in range(2):
                        v.scalar_tensor_tensor(
                            outs[g], in0s[g][:], scalar, in1s[g], AL.mult, AL.add
                        )

                # initial state lives in the pin tile: [gamma | ct0 | w0]
                cur_slab, cur_base = pint, F
                for k in range(N_RK4):
                    h = float(DTS[k])
                    curg = [g_state(cur_slab, cur_base, g) for g in range(2)]
                    X1 = head_eval_X(cur_slab, cur_base, "X1")
                    s0s, s02F, s0b = scratch(0)
                    head_stt(
                        [g_state(s0s, s0b, g) for g in range(2)], X1, -h / 2, curg
                    )
                    X2 = head_eval_X(s0s, s0b, "X2")
                    s1s, s12F, s1b = scratch(1)
                    head_stt(
                        [g_state(s1s, s1b, g) for g in range(2)], X2, -h / 2, curg
                    )
                    X3 = head_eval_X(s1s, s1b, "X3")
                    s2s, s22F, s2b = scratch(2)
                    head_stt(
                        [g_state(s2s, s2b, g) for g in range(2)], X3, -h, curg
                    )
                    X4 = head_eval_X(s2s, s2b, "X4")
                    A1 = [wpool.tile([P, 2, HW], F32, tag=f"A1{g}", name=f"A1{g}") for g in range(2)]
                    head_stt([a[:] for a in A1], X2, 2.0, [x[:] for x in X1])
                    A2 = [wpool.tile([P, 2, HW], F32, tag=f"A2{g}", name=f"A2{g}") for g in range(2)]
                    head_stt([a[:] for a in A2], X3, 2.0, [a[:] for a in A1])
                    A3 = [wpool.tile([P, 2, HW], F32, tag=f"A3{g}", name=f"A3{g}") for g in range(2)]
                    for g in range(2):
                        v.tensor_tensor(A3[g][:], A2[g][:], X4[g][:], AL.add)
                    ns, n2F, nb = slice2F(k)
                    head_stt(
                        [g_state(ns, nb, g) for g in range(2)], A3, -h / 6, curg
                    )
                    cur_slab, cur_base = ns, nb

                # --- midpoint (RK2) head intervals, same 2-group interleave ---
                for k in range(N_RK4, N_RK4 + N_MID):
                    h = float(DTS[k])
                    curg = [g_state(cur_slab, cur_base, g) for g in range(2)]
                    X1 = head_eval_X(cur_slab, cur_base, "X1")
                    s0s, s02F, s0b = scratch(0)
                    head_stt(
                        [g_state(s0s, s0b, g) for g in range(2)], X1, -h / 2, curg
                    )
                    X2 = head_eval_X2(s0s, s0b, h, "X2")
                    ns, n2F, nb = slice2F(k)
                    for g in range(2):
                        v.tensor_tensor(
                            g_state(ns, nb, g), curg[g], X2[g][:], AL.subtract
                        )
                    cur_slab, cur_base = ns, nb

                # --- seed interval K_SEED: midpoint step + Z init ---
                h = float(DTS[K_SEED])
                cur2F = cur_slab[:, cur_base : cur_base + 2 * F]
                Xp = eval_X(cur_slab, cur_base, AS[K_SEED], "Xp")  # (3/2)dt*X
                s0s, s02F, s0b = scratch(0)
                v.scalar_tensor_tensor(s02F, Xp[:], -THIRD, cur2F, AL.mult, AL.add)
                Xm = eval_X(s0s, s0b, h, "Xm")  # dt*X(mid)
                ns, n2F, nb = slice2F(K_SEED)
                v.tensor_tensor(n2F, cur2F, Xm[:], AL.subtract)
                Z = wpool.tile([P, 2 * F], F32, tag="Z")
                v.scalar_tensor_tensor(Z[:], Xp[:], THIRD, n2F, AL.mult, AL.add)
                cur_slab, cur_base = ns, nb

                def chunk_dma(k):
                    if k % CH == CH - 1:
                        c = k // CH
                        s = slabs[c % 2]
                        nc.sync.dma_start(
                            out=out[c], in_=s[:, F : F + CH * 2 * F]
                        )
                    elif k == NI - 2 and k // CH == (NI - 1) // CH:
                        # early part of the final partial chunk (all filled
                        # slices except the one the last interval writes)
                        c = (NI - 1) // CH
                        s = slabs[c % 2]
                        n_in = NI - c * CH
                        nc.sync.dma_start(
                            out=out[c][:, 0 : (n_in - 1) * 2 * F],
                            in_=s[:, F : F + (n_in - 1) * 2 * F],
                        )

                # --- Z-form AB2 mid-section (single full-width chain; the
                # off-path Z op fills the Y->v latency gap) ---
                for k in range(K_SEED + 1, K_EULER):
                    Xs = eval_X(cur_slab, cur_base, AS[k], "Xs")
                    ns, n2F, nb = slice2F(k)
                    v.tensor_tensor(n2F, Z[:], Xs[:], AL.subtract)
                    Z2 = wpool.tile([P, 2 * F], F32, tag="Z")
                    v.scalar_tensor_tensor(
                        Z2[:], Xs[:], -TWO_THIRD, Z[:], AL.mult, AL.add
                    )
                    Z = Z2
                    cur_slab, cur_base = ns, nb
                    chunk_dma(k)

                # --- multiplicative forward-Euler tail ---
                # State switches to [w | m] with m = -v; one Euler step is
                #   q = G + dtbar*w,  r = 1 + dtbar*m   (G = 1 - dtbar*gamma)
                #   w' = w*r,  m' = m*q
                # i.e. ONE stt  qr = state*dtbar + [G|1]  and ONE tt
                # state' = state * swap(qr).  Two half-width groups
                # interleaved: busy-bound ~510ns/interval, no exposed
                # latency hops.  (dtbar = const fp32 100/199; the ~1e-7
                # relative dt variation is far below the error budget.)
                # GONE = [G | 1] constant tile
                GONE = cpool.tile([P, 2 * F], F32, tag="gone")
                v.memset(GONE[:, F : 2 * F], 1.0)
                v.scalar_tensor_tensor(
                    GONE[:, 0:F],
                    pint[:, 0:F],
                    -DTBAR,
                    GONE[:, F : 2 * F],
                    AL.mult,
                    AL.add,
                )
                # transition: [ct|w] slice -> [w|m] tile (m = w - ct = -v)
                etr = cpool.tile([P, 2 * F], F32, tag="etrans")
                v.tensor_scalar_mul(
                    etr[:, 0:F],
                    cur_slab[:, cur_base + F : cur_base + 2 * F],
                    1.0,
                )
                v.tensor_tensor(
                    etr[:, F : 2 * F],
                    cur_slab[:, cur_base + F : cur_base + 2 * F],
                    cur_slab[:, cur_base : cur_base + F],
                    AL.subtract,
                )
                cur_slab, cur_base = etr, 0

                def qr_swap(qr_ap):
                    """[r | q] view of a [P, 2, HW] qr tile."""
                    return AP(
                        tensor=qr_ap.tensor,
                        offset=qr_ap.offset + HW,
                        ap=[list(qr_ap.ap[0]), [-HW, 2], [1, HW]],
                    )

                for k in range(K_EULER, NI):
                    curg = [g_state(cur_slab, cur_base, g) for g in range(2)]
                    qrs = []
                    for g in range(2):
                        qr = wpool.tile([P, 2, HW], F32, tag=f"qr{g}", name=f"qr{g}")
                        v.scalar_tensor_tensor(
                            qr[:],
                            curg[g],
                            DTBAR,
                            _two_block(GONE[:], F, sub=g * HW, width=HW),
                            AL.mult,
                            AL.add,
                        )
                        qrs.append(qr)
                    ns, n2F, nb = slice2F(k)
                    for g in range(2):
                        v.tensor_tensor(
                            g_state(ns, nb, g), curg[g], qr_swap(qrs[g][:]), AL.mult
                        )
                    cur_slab, cur_base = ns, nb
                    chunk_dma(k)
                # final partial chunk, in two pieces: everything except the
                # last slice goes out while the last interval still computes,
                # so only one 32KB transfer remains after the final op
                # (shorter end-of-kernel drain).
                last = NI - 1
                c = last // CH
                n_in = NI - c * CH
                if n_in > 0 and last % CH != CH - 1:
                    s = slabs[c % 2]
                    j0 = n_in - 1
                    nc.sync.dma_start(
                        out=out[c][:, j0 * 2 * F : n_in * 2 * F],
                        in_=s[:, F + j0 * 2 * F : F + n_in * 2 * F],
                    )

            if reps == 1:
                body()
            else:
                # timing mode: repeat the whole kernel body inside one NEFF so
                # per-rep HW time can be separated from dispatch overhead
                with tc.For_i(0, reps, 1):
                    body()
    # run_bass_via_pjrt does not finalize; Bacc needs it (register alloc +
    # sync-wait splitting happen in its compile() pipeline).
    nc.finalize()
    return nc


_NC_CACHE = {}


def _pack_inputs(params: np.ndarray) -> list:
    in_maps = []
    for c in range(N_CORES):
        sl = params[c * PER : (c + 1) * PER]
        pin = np.empty((P, 3 * F), dtype=np.float32)
        pin[:, 0:F] = sl[:, 1].reshape(P, F)  # gamma
        beta = sl[:, 0]
        pin[:, F : 2 * F] = (beta * (sl[:, 2] + sl[:, 3])).reshape(P, F)  # ct0
        pin[:, 2 * F : 3 * F] = (beta * sl[:, 2]).reshape(P, F)  # w0
        in_maps.append({"pin": pin})
    return in_maps


def kernel(params: np.ndarray) -> np.ndarray:
    params = np.asarray(params, dtype=np.float32)
    assert params.shape == (B, 4)

    if "nc" not in _NC_CACHE:
        _NC_CACHE["nc"] = build_nc()
    nc = _NC_CACHE["nc"]

    in_maps = _pack_inputs(params)
    res = run_bass_kernel_spmd(nc, in_maps, list(range(N_CORES)))

    out_full = np.empty((B, NUM_T, 3), dtype=np.float32)
    one = np.float32(1.0)
    S0 = params[:, 2]
    I0 = params[:, 3]
    out_full[:, 0, 0] = S0
    out_full[:, 0, 1] = I0
    out_full[:, 0, 2] = (one - S0) - I0
    for c in range(N_CORES):
        o = res.results[c]["out"]  # [NCHUNK, P, CH*2F]
        seq = o.reshape(NCHUNK, P, CH, 2, F).transpose(0, 2, 1, 3, 4)
        seq = seq.reshape(NCHUNK * CH, P, 2, F)[:NI]  # [NI, P, 2, F]
        c0 = seq[:, :, 0, :].reshape(NI, PER).T  # [PER, NI]
        c1 = seq[:, :, 1, :].reshape(NI, PER).T
        ib = one / params[c * PER : (c + 1) * PER, 0:1]  # 1/beta [PER,1]
        blk = out_full[c * PER : (c + 1) * PER]
        # intervals < K_EULER store [ct | w]; >= K_EULER store [w | m=-v]
        na = K_EULER
        S = c1[:, :na] * ib
        C = c0[:, :na] * ib
        blk[:, 1 : na + 1, 0] = S
        blk[:, 1 : na + 1, 1] = C - S
        blk[:, 1 : na + 1, 2] = one - C
        Se = c0[:, na:] * ib
        Ie = -c1[:, na:] * ib
        blk[:, na + 1 :, 0] = Se
        blk[:, na + 1 :, 1] = Ie
        blk[:, na + 1 :, 2] = (one - Se) - Ie
    return out_full


if __name__ == "__main__":
    rng = np.random.RandomState(0)
    p = rng.uniform(0, 1, (B, 4)).astype(np.float32)
    r = kernel(p)
    print(r.shape, r.dtype, r[0, :3], flush=True)


# revision 26
# speedup vs baseline: 1.2426x; 1.2426x over previous
"""SIR ODE batch integrator on 8 Trainium2 NeuronCores (Bass/Tile).

Problem: for each of B=65536 samples with params (beta, gamma, S0, I0),
integrate dS=-bSI, dI=bSI-gI, dR=gI over 199 fixed intervals
(t = linspace(0,100,200), fp32) and return the trajectory [B, 200, 3].

Strategy:
  - Pure data parallel: 8192 samples per core as [128 part, 64 free].
  - Scaled 2-state formulation: w = beta*S, ct = beta*C (C = S+I).
      dw/dt = -w*v,  dct/dt = -gamma*v,   v = ct - w  (= beta*I)
    so the derivative X = [gamma*v | w*v] is TWO plain elementwise ops
    (v = ct - w; X = (vv*a) * [gamma|w]) - no custom DVE op - and the
    state update is DIAGONAL (plain subtract).  Host recovers
    S = w/beta, I = v/beta, R = 1 - ct/beta.
  - Schedule (fp32-validated vs the reference: rel fro-norm 1.69e-3,
    gate 2e-2):
      k 0..3    RK4, two half-width sample groups interleaved so each
                group's op latency hides under the other group's ops
      k 4..7    midpoint RK2, same interleave
      k 8       midpoint step that seeds the AB2 history + Z init
      k 9..19   Z-form AB2, single full-width chain: with
                Xs_n = (3/2)dt_n*X_n and Z_n = Y_n + (1/3)Xs_{n-1},
                  Y_{n+1} = Z_n - Xs_n         (cycle: v -> Xs -> Y)
                  Z_{n+1} = Z_n - (2/3)Xs_n    (fills the Y->v gap)
      k 20..198 multiplicative forward Euler on state [w | m], m = -v:
                  qr = state*dtbar + [G|1],  state' = state*swap(qr)
                (G = 1 - dtbar*gamma).  TWO DVE ops per interval; two
                half-width groups interleaved -> busy-bound
                ~510ns/interval with no exposed latency hops.
  - gamma rides at column 0 of each staging slab so the X op can read
    [gamma | w] as a single two-block access pattern of one tensor.
  - Output: states are written in-place into [128, 64+16*128] staging
    slabs (2, ping-pong); one 1MB DMA per 16 intervals (13 DMAs total).
"""

import numpy as np

try:
    import concourse.bass as bass
except ImportError:  # pragma: no cover - container default location
    import sys

    sys.path.insert(0, "/opt/trn_rl_repo")
    import concourse.bass as bass

import concourse.bacc as bacc
import concourse.mybir as mybir
from concourse.ap import AP
from concourse.tile import TileContext
from concourse.bass_utils import run_bass_kernel_spmd

F32 = mybir.dt.float32
AL = mybir.AluOpType

N_CORES = 8
B = 65536
PER = B // N_CORES  # 8192 samples per core
P = 128
F = PER // P  # 64
NUM_T = 200
NI = NUM_T - 1  # 199 intervals
CH = 16  # intervals per output chunk (one DMA each)
NCHUNK = (NI + CH - 1) // CH  # 13 (last chunk has 7)
N_RK4 = 3  # RK4 head intervals
K_EULER = 20  # forward-Euler tail from this interval (validated: rel 1.69e-3)
N_MID = 4  # midpoint (RK2) head intervals after the RK4 block
K_SEED = N_RK4 + N_MID  # midpoint interval that seeds the AB2 history

# Bit-exact fp32 dt values of jnp.linspace(0, 100, 200, float32) diffs.
_DT_BITS = [
    0x3F00A4AA, 0x3F00A4AA, 0x3F00A4AA, 0x3F00A4AA, 0x3F00A4A8, 0x3F00A4AC, 0x3F00A4AC, 0x3F00A4A8, 0x3F00A4A8, 0x3F00A4A8,
    0x3F00A4B0, 0x3F00A4A8, 0x3F00A4A8, 0x3F00A4B0, 0x3F00A4A8, 0x3F00A4A8, 0x3F00A4B0, 0x3F00A4A0, 0x3F00A4B0, 0x3F00A4A0,
    0x3F00A4B0, 0x3F00A4B0, 0x3F00A4A0, 0x3F00A4B0, 0x3F00A4B0, 0x3F00A4A0, 0x3F00A4B0, 0x3F00A4B0, 0x3F00A4A0, 0x3F00A4B0,
    0x3F00A4A0, 0x3F00A4B0, 0x3F00A4A0, 0x3F00A4C0, 0x3F00A4A0, 0x3F00A4A0, 0x3F00A4C0, 0x3F00A4A0, 0x3F00A4A0, 0x3F00A4A0,
    0x3F00A4C0, 0x3F00A4A0, 0x3F00A4A0, 0x3F00A4C0, 0x3F00A4A0, 0x3F00A4A0, 0x3F00A4C0, 0x3F00A4A0, 0x3F00A4A0, 0x3F00A4C0,
    0x3F00A4A0, 0x3F00A4A0, 0x3F00A4C0, 0x3F00A4A0, 0x3F00A4A0, 0x3F00A4C0, 0x3F00A4A0, 0x3F00A4A0, 0x3F00A4A0, 0x3F00A4C0,
    0x3F00A4A0, 0x3F00A4A0, 0x3F00A4C0, 0x3F00A4A0, 0x3F00A4C0, 0x3F00A480, 0x3F00A4C0, 0x3F00A4C0, 0x3F00A480, 0x3F00A4C0,
    0x3F00A4C0, 0x3F00A480, 0x3F00A4C0, 0x3F00A4C0, 0x3F00A480, 0x3F00A4C0, 0x3F00A4C0, 0x3F00A480, 0x3F00A4C0, 0x3F00A480,
    0x3F00A4C0, 0x3F00A4C0, 0x3F00A480, 0x3F00A4C0, 0x3F00A4C0, 0x3F00A480, 0x3F00A4C0, 0x3F00A4C0, 0x3F00A480, 0x3F00A4C0,
    0x3F00A4C0, 0x3F00A480, 0x3F00A4C0, 0x3F00A4C0, 0x3F00A480, 0x3F00A4C0, 0x3F00A4C0, 0x3F00A480, 0x3F00A4C0, 0x3F00A4C0,
    0x3F00A480, 0x3F00A4C0, 0x3F00A4C0, 0x3F00A480, 0x3F00A4C0, 0x3F00A4C0, 0x3F00A480, 0x3F00A4C0, 0x3F00A4C0, 0x3F00A480,
    0x3F00A4C0, 0x3F00A4C0, 0x3F00A480, 0x3F00A4C0, 0x3F00A480, 0x3F00A4C0, 0x3F00A4C0, 0x3F00A480, 0x3F00A4C0, 0x3F00A4C0,
    0x3F00A480, 0x3F00A4C0, 0x3F00A4C0, 0x3F00A480, 0x3F00A4C0, 0x3F00A4C0, 0x3F00A480, 0x3F00A4C0, 0x3F00A480, 0x3F00A500,
    0x3F00A480, 0x3F00A480, 0x3F00A500, 0x3F00A480, 0x3F00A480, 0x3F00A500, 0x3F00A480, 0x3F00A480, 0x3F00A500, 0x3F00A480,
    0x3F00A480, 0x3F00A500, 0x3F00A480, 0x3F00A480, 0x3F00A500, 0x3F00A480, 0x3F00A480, 0x3F00A500, 0x3F00A480, 0x3F00A480,
    0x3F00A500, 0x3F00A480, 0x3F00A480, 0x3F00A500, 0x3F00A480, 0x3F00A480, 0x3F00A500, 0x3F00A480, 0x3F00A480, 0x3F00A480,
    0x3F00A500, 0x3F00A480, 0x3F00A480, 0x3F00A500, 0x3F00A480, 0x3F00A480, 0x3F00A500, 0x3F00A480, 0x3F00A480, 0x3F00A500,
    0x3F00A480, 0x3F00A480, 0x3F00A500, 0x3F00A480, 0x3F00A480, 0x3F00A500, 0x3F00A480, 0x3F00A480, 0x3F00A500, 0x3F00A480,
    0x3F00A480, 0x3F00A500, 0x3F00A480, 0x3F00A480, 0x3F00A500, 0x3F00A480, 0x3F00A480, 0x3F00A500, 0x3F00A480, 0x3F00A480,
    0x3F00A500, 0x3F00A480, 0x3F00A480, 0x3F00A500, 0x3F00A480, 0x3F00A480, 0x3F00A500, 0x3F00A480, 0x3F00A480,
]
DTS = np.array(_DT_BITS, dtype=np.uint32).view(np.float32)
assert DTS.shape == (NI,)

AS = [float(np.float32(1.5) * DTS[k]) for k in range(NI)]  # AB2 scale a_k
THIRD = float(np.float32(1.0 / 3.0))
TWO_THIRD = float(np.float32(2.0 / 3.0))
DTBAR = float(np.float32(np.float64(100.0) / 199.0))  # Euler-tail step

SLAB_COLS = F + CH * 2 * F  # gamma block + CH state slices


def _two_block(slab_ap, off2, sub=0, width=F):
    """AP reading [block at column sub | block at column off2+sub] of a slab
    (width columns each): free dims [[off2, 2], [1, width]]."""
    return AP(
        tensor=slab_ap.tensor,
        offset=slab_ap.offset + sub,
        ap=[list(slab_ap.ap[0]), [off2, 2], [1, width]],
    )


def _vv(v_ap):
    """[v | v] broadcast read of a [P, F] tile."""
    return v_ap.unsqueeze(1).broadcast_to([P, 2, F])


def _3d(ap2d):
    """View a [P, 2F] AP as [P, 2, F] (to match broadcast operands)."""
    return ap2d.rearrange("p (two f) -> p two f", two=2)


def build_nc(reps=1):
    # Bacc (not raw Bass): its compile() pipeline runs generate_event_semaphores,
    # which splits multi-wait sync conditions that TRN2 instructions can't carry.
    nc = bacc.Bacc(None)
    pin = nc.declare_dram_parameter("pin", [P, 3 * F], F32, isOutput=False)
    out = nc.declare_dram_parameter("out", [NCHUNK, P, CH * 2 * F], F32, isOutput=True)
    v = nc.vector

    with TileContext(nc) as tc:
        with (
            tc.tile_pool(name="const", bufs=1) as cpool,
            tc.tile_pool(name="slab", bufs=1) as spool,
            tc.tile_pool(name="work", bufs=2) as wpool,
        ):

            def body(_=None):
                pint = cpool.tile([P, 3 * F], F32, tag="pin")
                nc.sync.dma_start(out=pint[:], in_=pin[:])
                slabA = spool.tile([P, SLAB_COLS], F32, tag="slabA")
                slabB = spool.tile([P, SLAB_COLS], F32, tag="slabB")
                slabs = [slabA, slabB]
                # gamma block at column 0 of both slabs
                for s in slabs:
                    nc.sync.dma_start(out=s[:, 0:F], in_=pin[:, 0:F])

                def slice2F(k):
                    """State slice [ct | w] for interval k (2F wide)."""
                    s = slabs[(k // CH) % 2]
                    base = F + (k % CH) * 2 * F
                    return s, s[:, base : base + 2 * F], base

                def eval_X(src_slab, base, scale, xt_tag):
                    """v = ct - w; X = (vv*scale) * [gamma | w].  X layout
                    [X_ct | X_w] matching the [ct | w] state slices."""
                    vt = wpool.tile([P, F], F32, tag="v")
                    v.tensor_tensor(
                        vt[:],
                        src_slab[:, base : base + F],
                        src_slab[:, base + F : base + 2 * F],
                        AL.subtract,
                    )
                    xt = wpool.tile([P, 2 * F], F32, tag=xt_tag)
                    v.scalar_tensor_tensor(
                        _3d(xt[:]),
                        _vv(vt[:]),
                        scale,
                        _two_block(src_slab[:], base + F),
                        AL.mult,
                        AL.mult,
                    )
                    return xt

                # scratch slices in the OTHER slab (idle until chunk 1)
                def scratch(j):
                    s = slabs[1]
                    base = F + j * 2 * F
                    return s, s[:, base : base + 2 * F], base

                # --- head: RK4, intervals 0..N_RK4-1, two half-width groups
                # interleaved so dependency latency hides under the other
                # group's ops ---
                HW = F // 2  # 32 cols per group

                def g_state(slab_t, base, g):
                    """[ct_g | w_g] two-block view of a state slice."""
                    return _two_block(slab_t[:], F, sub=base + g * HW, width=HW)

                def g_gw(slab_t, base, g):
                    """[gamma_g | w_g] two-block view (gamma at slab col 0)."""
                    return _two_block(slab_t[:], base + F, sub=g * HW, width=HW)

                def head_eval_X(src_slab, base, tag):
                    return head_eval_X2(src_slab, base, 1.0, tag)

                def head_eval_X2(src_slab, base, scale, tag):
                    """Per-group derivative: returns [XA, XB] ([P,2,HW] tiles)."""
                    vts, xts = [], []
                    for g in range(2):
                        vt = wpool.tile([P, HW], F32, tag=f"v{g}")
                        v.tensor_tensor(
                            vt[:],
                            src_slab[:, base + g * HW : base + (g + 1) * HW],
                            src_slab[:, base + F + g * HW : base + F + (g + 1) * HW],
                            AL.subtract,
                        )
                        vts.append(vt)
                    for g in range(2):
                        xt = wpool.tile([P, 2, HW], F32, tag=f"{tag}{g}")
                        v.scalar_tensor_tensor(
                            xt[:],
                            vts[g][:].unsqueeze(1).broadcast_to([P, 2, HW]),
                            scale,
                            g_gw(src_slab, base, g),
                            AL.mult,
                            AL.mult,
                        )
                        xts.append(xt)
                    return xts

                def head_stt(outs, in0s, scalar, in1s):
                    for g in range(2):
                        v.scalar_tensor_tensor(
                            outs[g], in0s[g][:], scalar, in1s[g], AL.mult, AL.add
                        )

                # initial state lives in the pin tile: [gamma | ct0 | w0]
                cur_slab, cur_base = pint, F
                for k in range(N_RK4):
                    h = float(DTS[k])
                    curg = [g_state(cur_slab, cur_base, g) for g in range(2)]
                    X1 = head_eval_X(cur_slab, cur_base, "X1")
                    s0s, s02F, s0b = scratch(0)
                    head_stt(
                        [g_state(s0s, s0b, g) for g in range(2)], X1, -h / 2, curg
                    )
                    X2 = head_eval_X(s0s, s0b, "X2")
                    s1s, s12F, s1b = scratch(1)
                    head_stt(
                        [g_state(s1s, s1b, g) for g in range(2)], X2, -h / 2, curg
                    )
                    X3 = head_eval_X(s1s, s1b, "X3")
                    s2s, s22F, s2b = scratch(2)
                    head_stt(
                        [g_state(s2s, s2b, g) for g in range(2)], X3, -h, curg
                    )
                    X4 = head_eval_X(s2s, s2b, "X4")
                    A1 = [wpool.tile([P, 2, HW], F32, tag=f"A1{g}", name=f"A1{g}") for g in range(2)]
                    head_stt([a[:] for a in A1], X2, 2.0, [x[:] for x in X1])
                    A2 = [wpool.tile([P, 2, HW], F32, tag=f"A2{g}", name=f"A2{g}") for g in range(2)]
                    head_stt([a[:] for a in A2], X3, 2.0, [a[:] for a in A1])
                    A3 = [wpool.tile([P, 2, HW], F32, tag=f"A3{g}", name=f"A3{g}") for g in range(2)]
                    for g in range(2):
                        v.tensor_tensor(A3[g][:], A2[g][:], X4[g][:], AL.add)
                    ns, n2F, nb = slice2F(k)
                    head_stt(
                        [g_state(ns, nb, g) for g in range(2)], A3, -h / 6, curg
                    )
                    cur_slab, cur_base = ns, nb

                # --- midpoint (RK2) head intervals, same 2-group interleave ---
                for k in range(N_RK4, N_RK4 + N_MID):
                    h = float(DTS[k])
                    curg = [g_state(cur_slab, cur_base, g) for g in range(2)]
                    X1 = head_eval_X(cur_slab, cur_base, "X1")
                    s0s, s02F, s0b = scratch(0)
                    head_stt(
                        [g_state(s0s, s0b, g) for g in range(2)], X1, -h / 2, curg
                    )
                    X2 = head_eval_X2(s0s, s0b, h, "X2")
                    ns, n2F, nb = slice2F(k)
                    for g in range(2):
                        v.tensor_tensor(
                            g_state(ns, nb, g), curg[g], X2[g][:], AL.subtract
                        )
                    cur_slab, cur_base = ns, nb

                # --- seed interval K_SEED: midpoint step + Z init ---
                h = float(DTS[K_SEED])
                cur2F = cur_slab[:, cur_base : cur_base + 2 * F]
                Xp = eval_X(cur_slab, cur_base, AS[K_SEED], "Xp")  # (3/2)dt*X
                s0s, s02F, s0b = scratch(0)
                v.scalar_tensor_tensor(s02F, Xp[:], -THIRD, cur2F, AL.mult, AL.add)
                Xm = eval_X(s0s, s0b, h, "Xm")  # dt*X(mid)
                ns, n2F, nb = slice2F(K_SEED)
                v.tensor_tensor(n2F, cur2F, Xm[:], AL.subtract)
                Z = wpool.tile([P, 2 * F], F32, tag="Z")
                v.scalar_tensor_tensor(Z[:], Xp[:], THIRD, n2F, AL.mult, AL.add)
                cur_slab, cur_base = ns, nb

                def chunk_dma(k):
                    if k % CH == CH - 1:
                        c = k // CH
                        s = slabs[c % 2]
                        nc.sync.dma_start(
                            out=out[c], in_=s[:, F : F + CH * 2 * F]
                        )
                    elif k == NI - 2 and k // CH == (NI - 1) // CH:
                        # early part of the final partial chunk (all filled
                        # slices except the one the last interval writes)
                        c = (NI - 1) // CH
                        s = slabs[c % 2]
                        n_in = NI - c * CH
                        nc.sync.dma_start(
                            out=out[c][:, 0 : (n_in - 1) * 2 * F],
                            in_=s[:, F : F + (n_in - 1) * 2 * F],
                        )

                # --- Z-form AB2 mid-section (single full-width chain; the
                # off-path Z op fills the Y->v latency gap) ---
                for k in range(K_SEED + 1, K_EULER):
                    Xs = eval_X(cur_slab, cur_base, AS[k], "Xs")
                    ns, n2F, nb = slice2F(k)
                    v.tensor_tensor(n2F, Z[:], Xs[:], AL.subtract)
                    Z2 = wpool.tile([P, 2 * F], F32, tag="Z")
                    v.scalar_tensor_tensor(
                        Z2[:], Xs[:], -TWO_THIRD, Z[:], AL.mult, AL.add
                    )
                    Z = Z2
                    cur_slab, cur_base = ns, nb
                    chunk_dma(k)

                # --- multiplicative forward-Euler tail ---
                # State switches to [w | m] with m = -v; one Euler step is
                #   q = G + dtbar*w,  r = 1 + dtbar*m   (G = 1 - dtbar*gamma)
                #   w' = w*r,  m' = m*q
                # i.e. ONE stt  qr = state*dtbar + [G|1]  and ONE tt
                # state' = state * swap(qr).  Two half-width groups
                # interleaved: busy-bound ~510ns/interval, no exposed
                # latency hops.  (dtbar = const fp32 100/199; the ~1e-7
                # relative dt variation is far below the error budget.)
                # GONE = [G | 1] constant tile
                GONE = cpool.tile([P, 2 * F], F32, tag="gone")
                v.memset(GONE[:, F : 2 * F], 1.0)
                v.scalar_tensor_tensor(
                    GONE[:, 0:F],
                    pint[:, 0:F],
                    -DTBAR,
                    GONE[:, F : 2 * F],
                    AL.mult,
                    AL.add,
                )
                # transition: [ct|w] slice -> [w|m] tile (m = w - ct = -v)
                etr = cpool.tile([P, 2 * F], F32, tag="etrans")
                v.tensor_scalar_mul(
                    etr[:, 0:F],
                    cur_slab[:, cur_base + F : cur_base + 2 * F],
                    1.0,
                )
                v.tensor_tensor(
                    etr[:, F : 2 * F],
                    cur_slab[:, cur_base + F : cur_base + 2 * F],
                    cur_slab[:, cur_base : cur_base + F],
                    AL.subtract,
                )
                cur_slab, cur_base = etr, 0

                def qr_swap(qr_ap):
                    """[r | q] view of a [P, 2, HW] qr tile."""
                    return AP(
                        tensor=qr_ap.tensor,
                        offset=qr_ap.offset + HW,
                        ap=[list(qr_ap.ap[0]), [-HW, 2], [1, HW]],
                    )

                for k in range(K_EULER, NI):
                    curg = [g_state(cur_slab, cur_base, g) for g in range(2)]
                    qrs = []
                    for g in range(2):
                        qr = wpool.tile([P, 2, HW], F32, tag=f"qr{g}", name=f"qr{g}")
                        v.scalar_tensor_tensor(
                            qr[:],
                            curg[g],
                            DTBAR,
                            _two_block(GONE[:], F, sub=g * HW, width=HW),
                            AL.mult,
                            AL.add,
                        )
                        qrs.append(qr)
                    ns, n2F, nb = slice2F(k)
                    for g in range(2):
                        v.tensor_tensor(
                            g_state(ns, nb, g), curg[g], qr_swap(qrs[g][:]), AL.mult
                        )
                    cur_slab, cur_base = ns, nb
                    chunk_dma(k)
                # final partial chunk, in two pieces: everything except the
                # last slice goes out while the last interval still computes,
                # so only one 32KB transfer remains after the final op
                # (shorter end-of-kernel drain).
                last = NI - 1
                c = last // CH
                n_in = NI - c * CH
                if n_in > 0 and last % CH != CH - 1:
                    s = slabs[c % 2]
                    j0 = n_in - 1
                    nc.sync.dma_start(
                        out=out[c][:, j0 * 2 * F : n_in * 2 * F],
                        in_=s[:, F + j0 * 2 * F : F + n_in * 2 * F],
                    )

            if reps == 1:
                body()
            else:
                # timing mode: repeat the whole kernel body inside one NEFF so
                # per-rep HW time can be separated from dispatch overhead
                with tc.For_i(0, reps, 1):
                    body()
    # run_bass_via_pjrt does not finalize; Bacc needs it (register alloc +
    # sync-wait splitting happen in its compile() pipeline).
    nc.finalize()
    return nc


_NC_CACHE = {}


def _pack_inputs(params: np.ndarray) -> list:
    in_maps = []
    for c in range(N_CORES):
        sl = params[c * PER : (c + 1) * PER]
        pin = np.empty((P, 3 * F), dtype=np.float32)
        pin[:, 0:F] = sl[:, 1].reshape(P, F)  # gamma
        beta = sl[:, 0]
        pin[:, F : 2 * F] = (beta * (sl[:, 2] + sl[:, 3])).reshape(P, F)  # ct0
        pin[:, 2 * F : 3 * F] = (beta * sl[:, 2]).reshape(P, F)  # w0
        in_maps.append({"pin": pin})
    return in_maps


def kernel(params: np.ndarray) -> np.ndarray:
    params = np.asarray(params, dtype=np.float32)
    assert params.shape == (B, 4)

    if "nc" not in _NC_CACHE:
        _NC_CACHE["nc"] = build_nc()
    nc = _NC_CACHE["nc"]

    in_maps = _pack_inputs(params)
    res = run_bass_kernel_spmd(nc, in_maps, list(range(N_CORES)))

    out_full = np.empty((B, NUM_T, 3), dtype=np.float32)
    one = np.float32(1.0)
    S0 = params[:, 2]
    I0 = params[:, 3]
    out_full[:, 0, 0] = S0
    out_full[:, 0, 1] = I0
    out_full[:, 0, 2] = (one - S0) - I0
    for c in range(N_CORES):
        o = res.results[c]["out"]  # [NCHUNK, P, CH*2F]
        seq = o.reshape(NCHUNK, P, CH, 2, F).transpose(0, 2, 1, 3, 4)
        seq = seq.reshape(NCHUNK * CH, P, 2, F)[:NI]  # [NI, P, 2, F]
        c0 = seq[:, :, 0, :].reshape(NI, PER).T  # [PER, NI]
        c1 = seq[:, :, 1, :].reshape(NI, PER).T
        ib = one / params[c * PER : (c + 1) * PER, 0:1]  # 1/beta [PER,1]
        blk = out_full[c * PER : (c + 1) * PER]
        # intervals < K_EULER store [ct | w]; >= K_EULER store [w | m=-v]
        na = K_EULER
        S = c1[:, :na] * ib
        C = c0[:, :na] * ib
        blk[:, 1 : na + 1, 0] = S
        blk[:, 1 : na + 1, 1] = C - S
        blk[:, 1 : na + 1, 2] = one - C
        Se = c0[:, na:] * ib
        Ie = -c1[:, na:] * ib
        blk[:, na + 1 :, 0] = Se
        blk[:, na + 1 :, 1] = Ie
        blk[:, na + 1 :, 2] = (one - Se) - Ie
    return out_full


if __name__ == "__main__":
    rng = np.random.RandomState(0)
    p = rng.uniform(0, 1, (B, 4)).astype(np.float32)
    r = kernel(p)
    print(r.shape, r.dtype, r[0, :3], flush=True)


# revision 28
# speedup vs baseline: 1.8859x; 1.5177x over previous
"""SIR ODE batch integrator on 8 Trainium2 NeuronCores (Bass/Tile).

Problem: for each of B=65536 samples with params (beta, gamma, S0, I0),
integrate dS=-bSI, dI=bSI-gI, dR=gI over 199 fixed intervals
(t = linspace(0,100,200), fp32) and return the trajectory [B, 200, 3].

Strategy:
  - Pure data parallel: 8192 samples per core as [128 part, 64 free].
  - Scaled 2-state formulation: w = beta*S, ct = beta*C (C = S+I).
      dw/dt = -w*v,  dct/dt = -gamma*v,   v = ct - w  (= beta*I)
    so the derivative X = [gamma*v | w*v] is TWO plain elementwise ops
    (v = ct - w; X = (vv*a) * [gamma|w]) - no custom DVE op - and the
    state update is DIAGONAL (plain subtract).  Host recovers
    S = w/beta, I = v/beta, R = 1 - ct/beta.
  - Schedule (fp32-validated vs the reference: rel fro-norm 2.20e-3,
    gate 2e-2):
      k 0..2    RK4, two half-width sample groups interleaved so each
                group's op latency hides under the other group's ops
      k 3..6    midpoint RK2, same interleave
      k 7       midpoint step that seeds the AB2 history + Z init
      k 8..15   Z-form AB2, single full-width chain: with
                Xs_n = (3/2)dt_n*X_n and Z_n = Y_n + (1/3)Xs_{n-1},
                  Y_{n+1} = Z_n - Xs_n         (cycle: v -> Xs -> Y)
                  Z_{n+1} = Z_n - (2/3)Xs_n    (fills the Y->v gap)
      k 16..198 multiplicative forward Euler on state [w | m], m = -v:
                  qr = state*dtbar + [G|1],  state' = state*swap(qr)
                (G = 1 - dtbar*gamma).  TWO DVE ops per interval; two
                half-width groups interleaved -> busy-bound
                ~510ns/interval with no exposed latency hops.
  - gamma rides at column 0 of each staging slab so the X op can read
    [gamma | w] as a single two-block access pattern of one tensor.
  - Output: states are written in-place into [128, 64+16*128] staging
    slabs (2, ping-pong); one 1MB DMA per 16 intervals (13 DMAs total).
"""

import numpy as np

try:
    import concourse.bass as bass
except ImportError:  # pragma: no cover - container default location
    import sys

    sys.path.insert(0, "/opt/trn_rl_repo")
    import concourse.bass as bass

import concourse.bacc as bacc
import concourse.mybir as mybir
from concourse.ap import AP
from concourse.tile import TileContext
from concourse.bass_utils import run_bass_kernel_spmd

F32 = mybir.dt.float32
AL = mybir.AluOpType

N_CORES = 8
B = 65536
PER = B // N_CORES  # 8192 samples per core
P = 128
F = PER // P  # 64
NUM_T = 200
NI = NUM_T - 1  # 199 intervals
CH = 16  # intervals per output chunk (one DMA each)
NCHUNK = (NI + CH - 1) // CH  # 13 (last chunk has 7)
N_RK4 = 3  # RK4 head intervals
K_EULER = 16  # forward-Euler tail from this interval (validated: rel 2.20e-3)
N_MID = 4  # midpoint (RK2) head intervals after the RK4 block
K_SEED = N_RK4 + N_MID  # midpoint interval that seeds the AB2 history

# Bit-exact fp32 dt values of jnp.linspace(0, 100, 200, float32) diffs.
_DT_BITS = [
    0x3F00A4AA, 0x3F00A4AA, 0x3F00A4AA, 0x3F00A4AA, 0x3F00A4A8, 0x3F00A4AC, 0x3F00A4AC, 0x3F00A4A8, 0x3F00A4A8, 0x3F00A4A8,
    0x3F00A4B0, 0x3F00A4A8, 0x3F00A4A8, 0x3F00A4B0, 0x3F00A4A8, 0x3F00A4A8, 0x3F00A4B0, 0x3F00A4A0, 0x3F00A4B0, 0x3F00A4A0,
    0x3F00A4B0, 0x3F00A4B0, 0x3F00A4A0, 0x3F00A4B0, 0x3F00A4B0, 0x3F00A4A0, 0x3F00A4B0, 0x3F00A4B0, 0x3F00A4A0, 0x3F00A4B0,
    0x3F00A4A0, 0x3F00A4B0, 0x3F00A4A0, 0x3F00A4C0, 0x3F00A4A0, 0x3F00A4A0, 0x3F00A4C0, 0x3F00A4A0, 0x3F00A4A0, 0x3F00A4A0,
    0x3F00A4C0, 0x3F00A4A0, 0x3F00A4A0, 0x3F00A4C0, 0x3F00A4A0, 0x3F00A4A0, 0x3F00A4C0, 0x3F00A4A0, 0x3F00A4A0, 0x3F00A4C0,
    0x3F00A4A0, 0x3F00A4A0, 0x3F00A4C0, 0x3F00A4A0, 0x3F00A4A0, 0x3F00A4C0, 0x3F00A4A0, 0x3F00A4A0, 0x3F00A4A0, 0x3F00A4C0,
    0x3F00A4A0, 0x3F00A4A0, 0x3F00A4C0, 0x3F00A4A0, 0x3F00A4C0, 0x3F00A480, 0x3F00A4C0, 0x3F00A4C0, 0x3F00A480, 0x3F00A4C0,
    0x3F00A4C0, 0x3F00A480, 0x3F00A4C0, 0x3F00A4C0, 0x3F00A480, 0x3F00A4C0, 0x3F00A4C0, 0x3F00A480, 0x3F00A4C0, 0x3F00A480,
    0x3F00A4C0, 0x3F00A4C0, 0x3F00A480, 0x3F00A4C0, 0x3F00A4C0, 0x3F00A480, 0x3F00A4C0, 0x3F00A4C0, 0x3F00A480, 0x3F00A4C0,
    0x3F00A4C0, 0x3F00A480, 0x3F00A4C0, 0x3F00A4C0, 0x3F00A480, 0x3F00A4C0, 0x3F00A4C0, 0x3F00A480, 0x3F00A4C0, 0x3F00A4C0,
    0x3F00A480, 0x3F00A4C0, 0x3F00A4C0, 0x3F00A480, 0x3F00A4C0, 0x3F00A4C0, 0x3F00A480, 0x3F00A4C0, 0x3F00A4C0, 0x3F00A480,
    0x3F00A4C0, 0x3F00A4C0, 0x3F00A480, 0x3F00A4C0, 0x3F00A480, 0x3F00A4C0, 0x3F00A4C0, 0x3F00A480, 0x3F00A4C0, 0x3F00A4C0,
    0x3F00A480, 0x3F00A4C0, 0x3F00A4C0, 0x3F00A480, 0x3F00A4C0, 0x3F00A4C0, 0x3F00A480, 0x3F00A4C0, 0x3F00A480, 0x3F00A500,
    0x3F00A480, 0x3F00A480, 0x3F00A500, 0x3F00A480, 0x3F00A480, 0x3F00A500, 0x3F00A480, 0x3F00A480, 0x3F00A500, 0x3F00A480,
    0x3F00A480, 0x3F00A500, 0x3F00A480, 0x3F00A480, 0x3F00A500, 0x3F00A480, 0x3F00A480, 0x3F00A500, 0x3F00A480, 0x3F00A480,
    0x3F00A500, 0x3F00A480, 0x3F00A480, 0x3F00A500, 0x3F00A480, 0x3F00A480, 0x3F00A500, 0x3F00A480, 0x3F00A480, 0x3F00A480,
    0x3F00A500, 0x3F00A480, 0x3F00A480, 0x3F00A500, 0x3F00A480, 0x3F00A480, 0x3F00A500, 0x3F00A480, 0x3F00A480, 0x3F00A500,
    0x3F00A480, 0x3F00A480, 0x3F00A500, 0x3F00A480, 0x3F00A480, 0x3F00A500, 0x3F00A480, 0x3F00A480, 0x3F00A500, 0x3F00A480,
    0x3F00A480, 0x3F00A500, 0x3F00A480, 0x3F00A480, 0x3F00A500, 0x3F00A480, 0x3F00A480, 0x3F00A500, 0x3F00A480, 0x3F00A480,
    0x3F00A500, 0x3F00A480, 0x3F00A480, 0x3F00A500, 0x3F00A480, 0x3F00A480, 0x3F00A500, 0x3F00A480, 0x3F00A480,
]
DTS = np.array(_DT_BITS, dtype=np.uint32).view(np.float32)
assert DTS.shape == (NI,)

AS = [float(np.float32(1.5) * DTS[k]) for k in range(NI)]  # AB2 scale a_k
THIRD = float(np.float32(1.0 / 3.0))
TWO_THIRD = float(np.float32(2.0 / 3.0))
DTBAR = float(np.float32(np.float64(100.0) / 199.0))  # Euler-tail step

SLAB_COLS = F + CH * 2 * F  # gamma block + CH state slices


def _two_block(slab_ap, off2, sub=0, width=F):
    """AP reading [block at column sub | block at column off2+sub] of a slab
    (width columns each): free dims [[off2, 2], [1, width]]."""
    return AP(
        tensor=slab_ap.tensor,
        offset=slab_ap.offset + sub,
        ap=[list(slab_ap.ap[0]), [off2, 2], [1, width]],
    )


def _vv(v_ap):
    """[v | v] broadcast read of a [P, F] tile."""
    return v_ap.unsqueeze(1).broadcast_to([P, 2, F])


def _3d(ap2d):
    """View a [P, 2F] AP as [P, 2, F] (to match broadcast operands)."""
    return ap2d.rearrange("p (two f) -> p two f", two=2)


def build_nc(reps=1):
    # Bacc (not raw Bass): its compile() pipeline runs generate_event_semaphores,
    # which splits multi-wait sync conditions that TRN2 instructions can't carry.
    nc = bacc.Bacc(None)
    pin = nc.declare_dram_parameter("pin", [P, 3 * F], F32, isOutput=False)
    out = nc.declare_dram_parameter("out", [NCHUNK, P, CH * 2 * F], F32, isOutput=True)
    v = nc.vector

    with TileContext(nc) as tc:
        with (
            tc.tile_pool(name="const", bufs=1) as cpool,
            tc.tile_pool(name="slab", bufs=1) as spool,
            tc.tile_pool(name="work", bufs=2) as wpool,
        ):

            def body(_=None):
                pint = cpool.tile([P, 3 * F], F32, tag="pin")
                nc.sync.dma_start(out=pint[:], in_=pin[:])
                slabA = spool.tile([P, SLAB_COLS], F32, tag="slabA")
                slabB = spool.tile([P, SLAB_COLS], F32, tag="slabB")
                slabs = [slabA, slabB]
                # gamma block at column 0 of both slabs
                for s in slabs:
                    nc.sync.dma_start(out=s[:, 0:F], in_=pin[:, 0:F])

                def slice2F(k):
                    """State slice [ct | w] for interval k (2F wide)."""
                    s = slabs[(k // CH) % 2]
                    base = F + (k % CH) * 2 * F
                    return s, s[:, base : base + 2 * F], base

                def eval_X(src_slab, base, scale, xt_tag):
                    """v = ct - w; X = (vv*scale) * [gamma | w].  X layout
                    [X_ct | X_w] matching the [ct | w] state slices."""
                    vt = wpool.tile([P, F], F32, tag="v")
                    v.tensor_tensor(
                        vt[:],
                        src_slab[:, base : base + F],
                        src_slab[:, base + F : base + 2 * F],
                        AL.subtract,
                    )
                    xt = wpool.tile([P, 2 * F], F32, tag=xt_tag)
                    v.scalar_tensor_tensor(
                        _3d(xt[:]),
                        _vv(vt[:]),
                        scale,
                        _two_block(src_slab[:], base + F),
                        AL.mult,
                        AL.mult,
                    )
                    return xt

                # scratch slices in the OTHER slab (idle until chunk 1)
                def scratch(j):
                    s = slabs[1]
                    base = F + j * 2 * F
                    return s, s[:, base : base + 2 * F], base

                # --- head: RK4, intervals 0..N_RK4-1, two half-width groups
                # interleaved so dependency latency hides under the other
                # group's ops ---
                HW = F // 2  # 32 cols per group

                def g_state(slab_t, base, g):
                    """[ct_g | w_g] two-block view of a state slice."""
                    return _two_block(slab_t[:], F, sub=base + g * HW, width=HW)

                def g_gw(slab_t, base, g):
                    """[gamma_g | w_g] two-block view (gamma at slab col 0)."""
                    return _two_block(slab_t[:], base + F, sub=g * HW, width=HW)

                def head_eval_X(src_slab, base, tag):
                    return head_eval_X2(src_slab, base, 1.0, tag)

                def head_eval_X2(src_slab, base, scale, tag):
                    """Per-group derivative: returns [XA, XB] ([P,2,HW] tiles)."""
                    vts, xts = [], []
                    for g in range(2):
                        vt = wpool.tile([P, HW], F32, tag=f"v{g}")
                        v.tensor_tensor(
                            vt[:],
                            src_slab[:, base + g * HW : base + (g + 1) * HW],
                            src_slab[:, base + F + g * HW : base + F + (g + 1) * HW],
                            AL.subtract,
                        )
                        vts.append(vt)
                    for g in range(2):
                        xt = wpool.tile([P, 2, HW], F32, tag=f"{tag}{g}")
                        v.scalar_tensor_tensor(
                            xt[:],
                            vts[g][:].unsqueeze(1).broadcast_to([P, 2, HW]),
                            scale,
                            g_gw(src_slab, base, g),
                            AL.mult,
                            AL.mult,
                        )
                        xts.append(xt)
                    return xts

                def head_stt(outs, in0s, scalar, in1s):
                    for g in range(2):
                        v.scalar_tensor_tensor(
                            outs[g], in0s[g][:], scalar, in1s[g], AL.mult, AL.add
                        )

                # initial state lives in the pin tile: [gamma | ct0 | w0]
                cur_slab, cur_base = pint, F
                for k in range(N_RK4):
                    h = float(DTS[k])
                    curg = [g_state(cur_slab, cur_base, g) for g in range(2)]
                    X1 = head_eval_X(cur_slab, cur_base, "X1")
                    s0s, s02F, s0b = scratch(0)
                    head_stt(
                        [g_state(s0s, s0b, g) for g in range(2)], X1, -h / 2, curg
                    )
                    X2 = head_eval_X(s0s, s0b, "X2")
                    s1s, s12F, s1b = scratch(1)
                    head_stt(
                        [g_state(s1s, s1b, g) for g in range(2)], X2, -h / 2, curg
                    )
                    X3 = head_eval_X(s1s, s1b, "X3")
                    s2s, s22F, s2b = scratch(2)
                    head_stt(
                        [g_state(s2s, s2b, g) for g in range(2)], X3, -h, curg
                    )
                    X4 = head_eval_X(s2s, s2b, "X4")
                    A1 = [wpool.tile([P, 2, HW], F32, tag=f"A1{g}", name=f"A1{g}") for g in range(2)]
                    head_stt([a[:] for a in A1], X2, 2.0, [x[:] for x in X1])
                    A2 = [wpool.tile([P, 2, HW], F32, tag=f"A2{g}", name=f"A2{g}") for g in range(2)]
                    head_stt([a[:] for a in A2], X3, 2.0, [a[:] for a in A1])
                    A3 = [wpool.tile([P, 2, HW], F32, tag=f"A3{g}", name=f"A3{g}") for g in range(2)]
                    for g in range(2):
                        v.tensor_tensor(A3[g][:], A2[g][:], X4[g][:], AL.add)
                    ns, n2F, nb = slice2F(k)
                    head_stt(
                        [g_state(ns, nb, g) for g in range(2)], A3, -h / 6, curg
                    )
                    cur_slab, cur_base = ns, nb

                # --- midpoint (RK2) head intervals, same 2-group interleave ---
                for k in range(N_RK4, N_RK4 + N_MID):
                    h = float(DTS[k])
                    curg = [g_state(cur_slab, cur_base, g) for g in range(2)]
                    X1 = head_eval_X(cur_slab, cur_base, "X1")
                    s0s, s02F, s0b = scratch(0)
                    head_stt(
                        [g_state(s0s, s0b, g) for g in range(2)], X1, -h / 2, curg
                    )
                    X2 = head_eval_X2(s0s, s0b, h, "X2")
                    ns, n2F, nb = slice2F(k)
                    for g in range(2):
                        v.tensor_tensor(
                            g_state(ns, nb, g), curg[g], X2[g][:], AL.subtract
                        )
                    cur_slab, cur_base = ns, nb

                # --- seed interval K_SEED: midpoint step + Z init ---
                h = float(DTS[K_SEED])
                cur2F = cur_slab[:, cur_base : cur_base + 2 * F]
                Xp = eval_X(cur_slab, cur_base, AS[K_SEED], "Xp")  # (3/2)dt*X
                s0s, s02F, s0b = scratch(0)
                v.scalar_tensor_tensor(s02F, Xp[:], -THIRD, cur2F, AL.mult, AL.add)
                Xm = eval_X(s0s, s0b, h, "Xm")  # dt*X(mid)
                ns, n2F, nb = slice2F(K_SEED)
                v.tensor_tensor(n2F, cur2F, Xm[:], AL.subtract)
                Z = wpool.tile([P, 2 * F], F32, tag="Z")
                v.scalar_tensor_tensor(Z[:], Xp[:], THIRD, n2F, AL.mult, AL.add)
                cur_slab, cur_base = ns, nb

                def chunk_dma(k):
                    if k % CH == CH - 1:
                        c = k // CH
                        s = slabs[c % 2]
                        nc.sync.dma_start(
                            out=out[c], in_=s[:, F : F + CH * 2 * F]
                        )
                    elif k == NI - 2 and k // CH == (NI - 1) // CH:
                        # early part of the final partial chunk (all filled
                        # slices except the one the last interval writes)
                        c = (NI - 1) // CH
                        s = slabs[c % 2]
                        n_in = NI - c * CH
                        nc.sync.dma_start(
                            out=out[c][:, 0 : (n_in - 1) * 2 * F],
                            in_=s[:, F : F + (n_in - 1) * 2 * F],
                        )

                # --- Z-form AB2 mid-section (single full-width chain; the
                # off-path Z op fills the Y->v latency gap) ---
                for k in range(K_SEED + 1, K_EULER):
                    Xs = eval_X(cur_slab, cur_base, AS[k], "Xs")
                    ns, n2F, nb = slice2F(k)
                    v.tensor_tensor(n2F, Z[:], Xs[:], AL.subtract)
                    Z2 = wpool.tile([P, 2 * F], F32, tag="Z")
                    v.scalar_tensor_tensor(
                        Z2[:], Xs[:], -TWO_THIRD, Z[:], AL.mult, AL.add
                    )
                    Z = Z2
                    cur_slab, cur_base = ns, nb
                    chunk_dma(k)

                # --- multiplicative forward-Euler tail ---
                # State switches to [w | m] with m = -v; one Euler step is
                #   q = G + dtbar*w,  r = 1 + dtbar*m   (G = 1 - dtbar*gamma)
                #   w' = w*r,  m' = m*q
                # i.e. ONE stt  qr = state*dtbar + [G|1]  and ONE tt
                # state' = state * swap(qr).  Two half-width groups
                # interleaved: busy-bound ~510ns/interval, no exposed
                # latency hops.  (dtbar = const fp32 100/199; the ~1e-7
                # relative dt variation is far below the error budget.)
                # GONE = [G | 1] constant tile
                GONE = cpool.tile([P, 2 * F], F32, tag="gone")
                v.memset(GONE[:, F : 2 * F], 1.0)
                v.scalar_tensor_tensor(
                    GONE[:, 0:F],
                    pint[:, 0:F],
                    -DTBAR,
                    GONE[:, F : 2 * F],
                    AL.mult,
                    AL.add,
                )
                # transition: [ct|w] slice -> [w|m] tile (m = w - ct = -v)
                etr = cpool.tile([P, 2 * F], F32, tag="etrans")
                v.tensor_scalar_mul(
                    etr[:, 0:F],
                    cur_slab[:, cur_base + F : cur_base + 2 * F],
                    1.0,
                )
                v.tensor_tensor(
                    etr[:, F : 2 * F],
                    cur_slab[:, cur_base + F : cur_base + 2 * F],
                    cur_slab[:, cur_base : cur_base + F],
                    AL.subtract,
                )
                cur_slab, cur_base = etr, 0

                def qr_swap(qr_ap):
                    """[r | q] view of a [P, 2, HW] qr tile."""
                    return AP(
                        tensor=qr_ap.tensor,
                        offset=qr_ap.offset + HW,
                        ap=[list(qr_ap.ap[0]), [-HW, 2], [1, HW]],
                    )

                for k in range(K_EULER, NI):
                    curg = [g_state(cur_slab, cur_base, g) for g in range(2)]
                    qrs = []
                    for g in range(2):
                        qr = wpool.tile([P, 2, HW], F32, tag=f"qr{g}", name=f"qr{g}")
                        v.scalar_tensor_tensor(
                            qr[:],
                            curg[g],
                            DTBAR,
                            _two_block(GONE[:], F, sub=g * HW, width=HW),
                            AL.mult,
                            AL.add,
                        )
                        qrs.append(qr)
                    ns, n2F, nb = slice2F(k)
                    for g in range(2):
                        v.tensor_tensor(
                            g_state(ns, nb, g), curg[g], qr_swap(qrs[g][:]), AL.mult
                        )
                    cur_slab, cur_base = ns, nb
                    chunk_dma(k)
                # final partial chunk, in two pieces: everything except the
                # last slice goes out while the last interval still computes,
                # so only one 32KB transfer remains after the final op
                # (shorter end-of-kernel drain).
                last = NI - 1
                c = last // CH
                n_in = NI - c * CH
                if n_in > 0 and last % CH != CH - 1:
                    s = slabs[c % 2]
                    j0 = n_in - 1
                    nc.sync.dma_start(
                        out=out[c][:, j0 * 2 * F : n_in * 2 * F],
                        in_=s[:, F + j0 * 2 * F : F + n_in * 2 * F],
                    )

            if reps == 1:
                body()
            else:
                # timing mode: repeat the whole kernel body inside one NEFF so
                # per-rep HW time can be separated from dispatch overhead
                with tc.For_i(0, reps, 1):
                    body()
    # run_bass_via_pjrt does not finalize; Bacc needs it (register alloc +
    # sync-wait splitting happen in its compile() pipeline).
    nc.finalize()
    return nc


_NC_CACHE = {}


def _pack_inputs(params: np.ndarray) -> list:
    in_maps = []
    for c in range(N_CORES):
        sl = params[c * PER : (c + 1) * PER]
        pin = np.empty((P, 3 * F), dtype=np.float32)
        pin[:, 0:F] = sl[:, 1].reshape(P, F)  # gamma
        beta = sl[:, 0]
        pin[:, F : 2 * F] = (beta * (sl[:, 2] + sl[:, 3])).reshape(P, F)  # ct0
        pin[:, 2 * F : 3 * F] = (beta * sl[:, 2]).reshape(P, F)  # w0
        in_maps.append({"pin": pin})
    return in_maps


def kernel(params: np.ndarray) -> np.ndarray:
    params = np.asarray(params, dtype=np.float32)
    assert params.shape == (B, 4)

    if "nc" not in _NC_CACHE:
        _NC_CACHE["nc"] = build_nc()
    nc = _NC_CACHE["nc"]

    in_maps = _pack_inputs(params)
    res = run_bass_kernel_spmd(nc, in_maps, list(range(N_CORES)))

    out_full = np.empty((B, NUM_T, 3), dtype=np.float32)
    one = np.float32(1.0)
    S0 = params[:, 2]
    I0 = params[:, 3]
    out_full[:, 0, 0] = S0
    out_full[:, 0, 1] = I0
    out_full[:, 0, 2] = (one - S0) - I0
    for c in range(N_CORES):
        o = res.results[c]["out"]  # [NCHUNK, P, CH*2F]
        seq = o.reshape(NCHUNK, P, CH, 2, F).transpose(0, 2, 1, 3, 4)
        seq = seq.reshape(NCHUNK * CH, P, 2, F)[:NI]  # [NI, P, 2, F]
        c0 = seq[:, :, 0, :].reshape(NI, PER).T  # [PER, NI]
        c1 = seq[:, :, 1, :].reshape(NI, PER).T
        ib = one / params[c * PER : (c + 1) * PER, 0:1]  # 1/beta [PER,1]
        blk = out_full[c * PER : (c + 1) * PER]
        # intervals < K_EULER store [ct | w]; >= K_EULER store [w | m=-v]
        na = K_EULER
        S = c1[:, :na] * ib
        C = c0[:, :na] * ib
        blk[:, 1 : na + 1, 0] = S
        blk[:, 1 : na + 1, 1] = C - S
        blk[:, 1 : na + 1, 2] = one - C
        Se = c0[:, na:] * ib
        Ie = -c1[:, na:] * ib
        blk[:, na + 1 :, 0] = Se
        blk[:, na + 1 :, 1] = Ie
        blk[:, na + 1 :, 2] = (one - Se) - Ie
    return out_full


if __name__ == "__main__":
    rng = np.random.RandomState(0)
    p = rng.uniform(0, 1, (B, 4)).astype(np.float32)
    r = kernel(p)
    print(r.shape, r.dtype, r[0, :3], flush=True)
